# revision 9
# baseline (speedup 1.0000x reference)
"""GCN (4-layer message-passing) Trainium2 kernel, 8-core SPMD.

Math (matches PyG GCNConv with self-loops, per reference):
    deg[d]  = in-degree over (edges + self-loops)
    dinv    = deg^-1/2
    h0      = x @ W_emb + b_emb
    layer i: h <- tanh( dinv[d] * sum_{e: dst=d} dinv[src_e] * (h @ W_i)[src_e] + b_i )
    out     = h @ W_out + b_out

Distribution: nodes sharded across 8 cores (dst-sharded edges). Per layer:
  1. transform own shard:  hWd = dinv * (h @ W)   (PE matmul + ACT scale/cast bf16)
  2. AllGather hWd across cores in TWO row-pieces (piece 0 = blocks 0..24,
     piece 1 = blocks 25..48), so gathers of piece 0 overlap the piece-1
     collective, and the next layer's collectives launch mid-aggregation.
  3. dma_gather (SWDGE) each in-edge's source row, sorted by (dst blk, piece)
  4. segment-sum via PE matmuls against one-hot selection tiles built
     ON-CHIP (DVE is_equal against an iota ramp), accumulated in PSUM
  5. + self-loop term (DVE per-partition multiply of local hwd) [+ bias]
  6. tanh with per-partition dinv scale on ACT; PE-transpose back to h^T;
     immediately transform for the next layer and feed its AllGather.

The embedding layer is folded into layer 1's weights host-side.
Each AG piece's table has < 32768 rows so int16 gather indices address it
directly (no lo/hi split).
"""

import math

import ml_dtypes
import numpy as np

BF16 = ml_dtypes.bfloat16
P = 128

CFG_FULL = dict(N=50000, E=800000, DIN=128, DH=256, DOUT=64, L=4, NC=8)

CHUNK_BLOCKS = 2     # dst blocks per gather/aggregation chunk
STEP = 24            # max tiles (x128 idxs) per dma_gather call
PIECE0_NB = 25       # blocks 0..24 -> AG piece 0; rest -> piece 1
SCRATCH = 49152


def kernel(**inputs) -> np.ndarray:
    out, _ = run(inputs, CFG_FULL)
    return out


# ----------------------------------------------------------------------------
# host-side preprocessing
# ----------------------------------------------------------------------------


def _ceil_div(a, b):
    return (a + b - 1) // b


def preprocess(inputs, cfg):
    N, E, DIN, DH, DOUT, L, NC = (
        cfg["N"], cfg["E"], cfg["DIN"], cfg["DH"], cfg["DOUT"], cfg["L"], cfg["NC"],
    )
    x = np.asarray(inputs["x"], np.float32)
    ei = np.asarray(inputs["edge_index"]).astype(np.int64)
    W_emb = np.asarray(inputs["W_emb"], np.float32)
    b_emb = np.asarray(inputs["b_emb"], np.float32)
    W_conv = np.asarray(inputs["W_conv"], np.float32)
    b_conv = np.asarray(inputs["b_conv"], np.float32)
    W_out = np.asarray(inputs["W_out"], np.float32)
    b_out = np.asarray(inputs["b_out"], np.float32)

    deg = (np.bincount(ei[1], minlength=N) + 1).astype(np.float32)
    dinv = (1.0 / np.sqrt(np.maximum(deg, 1.0))).astype(np.float32)
    sqdeg = np.sqrt(np.maximum(deg, 1.0)).astype(np.float32)

    # self-edges (incl. the implicit self-loop): exact integer multiplicity
    # applied on-chip as a per-partition DVE multiply of the local hwd rows
    selfmask = ei[0] == ei[1]
    selfk = 1 + np.bincount(ei[1][selfmask], minlength=N)
    src = ei[0][~selfmask]
    dst = ei[1][~selfmask]

    NPs = _ceil_div(N, NC)          # real nodes per shard (6250)
    NB = _ceil_div(NPs, P)          # dst blocks per core (49)
    NPP = NB * P                    # padded nodes per shard (6272)
    NB0 = PIECE0_NB
    NB1 = NB - NB0
    PROWS = [NB0 * P, NB1 * P]      # piece rows per core
    AGR = [NC * PROWS[0], NC * PROWS[1]]
    assert max(AGR) < 32768

    # edge -> (piece, piece-local AG row)
    cs = src // NPs
    ls = src - cs * NPs
    piece = (ls >= PROWS[0]).astype(np.int64)
    prow = np.where(piece == 0, cs * PROWS[0] + ls,
                    cs * PROWS[1] + (ls - PROWS[0]))

    core_of = dst // NPs
    d_loc = dst - core_of * NPs
    blk = d_loc // P
    col = d_loc % P

    # per-core edge partitions, sorted by (block, piece, dstcol, srcrow)
    cores = []
    nseg = np.zeros((NC, NB, 2), np.int64)
    for c in range(NC):
        m = core_of == c
        a_blk, a_pc, a_col, a_row = blk[m], piece[m], col[m], prow[m]
        order = np.lexsort((a_row, a_col, a_pc, a_blk))
        a_blk, a_pc, a_col, a_row = (
            a_blk[order], a_pc[order], a_col[order], a_row[order],
        )
        cnt = np.bincount(a_blk * 2 + a_pc, minlength=NB * 2).reshape(NB, 2)
        nseg[c] = cnt
        cores.append((a_blk, a_pc, a_col, a_row))

    nmax = nseg.max(axis=0)                      # [NB, 2]
    T = _ceil_div(nmax, P)                       # tiles per (block, piece)

    # chunk layout (identical across cores)
    chunks = []
    gidx_col = 0
    tile_ctr = 0
    for g0 in range(0, NB, CHUNK_BLOCKS):
        blocks = list(range(g0, min(g0 + CHUNK_BLOCKS, NB)))
        tp0 = int(T[blocks, 0].sum())
        tp1 = int(T[blocks, 1].sum())
        cols0 = (gidx_col, gidx_col + tp0 * P // 16)
        gidx_col = cols0[1]
        cols1 = (gidx_col, gidx_col + tp1 * P // 16)
        gidx_col = cols1[1]
        base0, base1 = {}, {}
        t = 0
        for b in blocks:
            base0[b] = t
            t += int(T[b, 0])
        t = 0
        for b in blocks:
            base1[b] = t
            t += int(T[b, 1])
        smat_tiles = (tile_ctr, tile_ctr + tp0 + tp1)
        tile_ctr = smat_tiles[1]
        chunks.append(dict(
            blocks=blocks, tp0=tp0, tp1=tp1, cols0=cols0, cols1=cols1,
            base0=base0, base1=base1, smat_tiles=smat_tiles,
        ))
    GC = gidx_col
    TT = tile_ctr

    meta = dict(
        NPs=NPs, NB=NB, NPP=NPP, NB0=NB0, NB1=NB1, PROWS=PROWS, AGR=AGR,
        T=T, chunks=chunks, GC=GC, TT=TT,
        hb_emb=bool(np.any(b_emb @ W_conv[0])),
        hb_conv=bool(np.any(b_conv)),
        hb_out=bool(np.any(b_out)),
    )

    # shared weights
    W1x = (W_emb @ W_conv[0]).astype(BF16)                    # [DIN, DH]
    bemb1 = (b_emb @ W_conv[0]).reshape(1, DH).astype(BF16)
    Wc = W_conv[1:].reshape((L - 1) * DH, DH).astype(BF16) if L > 1 else \
        np.zeros((0, DH), BF16)
    bc = b_conv.reshape(1, L * DH).astype(BF16)
    Wo = W_out.astype(BF16)                                    # [DH, DOUT]
    bo = b_out.reshape(1, DOUT).astype(BF16)

    in_maps = []
    for c in range(NC):
        a_blk, a_pc, a_col, a_row = cores[c]
        n0 = c * NPs
        n1 = min(n0 + NPs, N)
        nreal = n1 - n0

        # per-edge slot in the chunk-ordered tile stream
        seg_id = a_blk * 2 + a_pc
        seg_start = np.zeros(NB * 2, np.int64)
        cnts = np.bincount(seg_id, minlength=NB * 2)
        seg_start[1:] = np.cumsum(cnts)[:-1]
        epos = np.arange(len(seg_id)) - seg_start[seg_id]

        tile_of_seg = np.zeros(NB * 2, np.int64)
        for ch in chunks:
            for b in ch["blocks"]:
                tile_of_seg[b * 2] = ch["smat_tiles"][0] + ch["base0"][b]
                tile_of_seg[b * 2 + 1] = (
                    ch["smat_tiles"][0] + ch["tp0"] + ch["base1"][b]
                )
        e_tile = tile_of_seg[seg_id] + epos // P
        e_row = epos % P

        # dst-column code per (tile, slot); -1 marks padding
        colcode = np.full((P, TT), -1, np.int16)
        colcode[e_row, e_tile] = a_col.astype(np.int16)

        # gather indices, wrapped layout [16->128, GC] int16
        gidx = np.zeros((16, GC), np.int16)
        for ch in chunks:
            for p, colrange, base_map, tcount in (
                (0, ch["cols0"], ch["base0"], ch["tp0"]),
                (1, ch["cols1"], ch["base1"], ch["tp1"]),
            ):
                if tcount == 0:
                    continue
                vals = np.zeros(tcount * P, np.int64)
                for b in ch["blocks"]:
                    m = (a_blk == b) & (a_pc == p)
                    v = a_row[m]
                    off = base_map[b] * P
                    vals[off:off + len(v)] = v
                c0, c1 = colrange
                gidx[:, c0:c1] = vals.reshape(c1 - c0, 16).T
        gidx = np.tile(gidx, (8, 1)).astype(np.int16)

        # dinv [128, NB] fp32 ; selfk [128, NB] fp32 ; sqdeg [1, NPP] bf16
        dl = np.ones(NPP, np.float32)
        dl[:nreal] = dinv[n0:n1]
        dinvp = dl.reshape(NB, P).T.copy()
        kk = np.zeros(NPP, np.float32)
        kk[:nreal] = selfk[n0:n1]
        selfkp = kk.reshape(NB, P).T.copy()
        sq = np.ones(NPP, np.float32)
        sq[:nreal] = sqdeg[n0:n1]
        sqdegp = sq.reshape(1, NPP).astype(BF16)

        xT = np.zeros((DIN, NPP), BF16)
        xT[:, :nreal] = x[n0:n1].T

        in_maps.append(dict(
            xT=np.ascontiguousarray(xT),
            gidx=np.ascontiguousarray(gidx),
            colcode=np.ascontiguousarray(colcode),
            dinvp=np.ascontiguousarray(dinvp),
            selfkp=np.ascontiguousarray(selfkp),
            sqdegp=np.ascontiguousarray(sqdegp),
            w1x=W1x, bemb1=bemb1, wc=Wc, bc=bc, wo=Wo, bo=bo,
        ))

    return in_maps, meta


# ----------------------------------------------------------------------------
# device program
# ----------------------------------------------------------------------------


def build_program(meta, cfg):
    import concourse.bacc as bacc
    import concourse.mybir as mybir
    import concourse.tile as tile
    from concourse.masks import make_identity

    N, DIN, DH, DOUT, L, NC = (
        cfg["N"], cfg["DIN"], cfg["DH"], cfg["DOUT"], cfg["L"], cfg["NC"],
    )
    NPs, NB, NPP, NB0, PROWS, AGR = (
        meta["NPs"], meta["NB"], meta["NPP"], meta["NB0"], meta["PROWS"],
        meta["AGR"],
    )
    T, chunks, GC, TT = meta["T"], meta["chunks"], meta["GC"], meta["TT"]
    hb_emb, hb_conv, hb_out = meta["hb_emb"], meta["hb_conv"], meta["hb_out"]
    assert DIN == P

    f32 = mybir.dt.float32
    bf16 = mybir.dt.bfloat16
    i16 = mybir.dt.int16
    TANH = mybir.ActivationFunctionType.Tanh
    COPY = mybir.ActivationFunctionType.Copy
    EQ = mybir.AluOpType.is_equal
    MUL = mybir.AluOpType.mult
    ADD = mybir.AluOpType.add

    nc = bacc.Bacc("TRN2", target_bir_lowering=False, debug=False, num_devices=NC,
                   num_swdge_queues=4, dynamic_dma_scratch_size=SCRATCH)

    # I/O
    d_xT = nc.dram_tensor("xT", [DIN, NPP], bf16, kind="ExternalInput")
    d_gidx = nc.dram_tensor("gidx", [P, GC], i16, kind="ExternalInput")
    d_colc = nc.dram_tensor("colcode", [P, TT], i16, kind="ExternalInput")
    d_dinv = nc.dram_tensor("dinvp", [P, NB], f32, kind="ExternalInput")
    d_selfk = nc.dram_tensor("selfkp", [P, NB], f32, kind="ExternalInput")
    d_sqdeg = nc.dram_tensor("sqdegp", [1, NPP], bf16, kind="ExternalInput")
    d_w1x = nc.dram_tensor("w1x", [DIN, DH], bf16, kind="ExternalInput")
    d_bemb1 = nc.dram_tensor("bemb1", [1, DH], bf16, kind="ExternalInput")
    d_wc = nc.dram_tensor("wc", [(L - 1) * DH, DH], bf16, kind="ExternalInput")
    d_bc = nc.dram_tensor("bc", [1, L * DH], bf16, kind="ExternalInput")
    d_wo = nc.dram_tensor("wo", [DH, DOUT], bf16, kind="ExternalInput")
    d_bo = nc.dram_tensor("bo", [1, DOUT], bf16, kind="ExternalInput")
    d_out = nc.dram_tensor("out", [NPs, DOUT], f32, kind="ExternalOutput")

    with tile.TileContext(nc) as tc:
        pers = tc.alloc_tile_pool(name="pers", bufs=1)
        dpool = tc.alloc_tile_pool(name="dpers", bufs=1, space="DRAM")

        # internal DRAM for the collectives: [layer][piece]
        agin = [
            [dpool.tile([PROWS[p], DH], bf16, name=f"agin{i}_{p}",
                        tag=f"agin{i}_{p}") for p in range(2)]
            for i in range(L)
        ]
        agout = [
            [dpool.tile([AGR[p], DH], bf16, name=f"agout{i}_{p}",
                        tag=f"agout{i}_{p}", addr_space="Shared")
             for p in range(2)]
            for i in range(L)
        ]

        def stile(shape, dtype, name):
            return pers.tile(shape, dtype, name=name, tag=name)

        # persistent SBUF state
        hT0 = stile([P, NPP], bf16, "hT0_sb")
        hT1 = stile([P, NPP], bf16, "hT1_sb")
        hwdall = stile([P, NB * DH], bf16, "hwdall_sb")
        gidx = stile([P, GC], i16, "gidx_sb")
        colc = stile([P, TT], i16, "colc_sb")
        dinv = stile([P, NB], f32, "dinv_sb")
        selfk = stile([P, NB], f32, "selfk_sb")
        w1x = stile([DIN, DH], bf16, "w1x_sb")
        wc = stile([P, 2 * (L - 1) * DH], bf16, "wc_sb")
        wo = stile([P, 2 * DOUT], bf16, "wo_sb")
        ident = stile([P, P], bf16, "ident_sb")
        iota = stile([P, P], i16, "iota_sb")

        nc.sync.dma_start(out=gidx[:], in_=d_gidx[:])
        nc.sync.dma_start(out=colc[:], in_=d_colc[:])
        nc.sync.dma_start(out=dinv[:], in_=d_dinv[:])
        nc.sync.dma_start(out=selfk[:], in_=d_selfk[:])
        nc.sync.dma_start(out=w1x[:], in_=d_w1x[:])
        for i in range(L - 1):
            for k in range(2):
                nc.sync.dma_start(
                    out=wc[:, (2 * i + k) * DH:(2 * i + k + 1) * DH],
                    in_=d_wc[i * DH + k * P:i * DH + (k + 1) * P, :],
                )
        for k in range(2):
            nc.sync.dma_start(
                out=wo[:, k * DOUT:(k + 1) * DOUT],
                in_=d_wo[k * P:(k + 1) * P, :],
            )
        make_identity(nc, ident[:])
        # iota row 0..127 along the free dim, same on every partition
        nc.gpsimd.iota(iota[:], pattern=[[1, P]], base=0, channel_multiplier=0)
        any_bias = hb_emb or hb_conv or hb_out
        if any_bias:
            sqd = stile([1, NPP], bf16, "sqdeg_sb")
            bemb1 = stile([1, DH], bf16, "bemb1_sb")
            bc = stile([1, L * DH], bf16, "bc_sb")
            bo = stile([1, DOUT], bf16, "bo_sb")
            ones = stile([1, P], bf16, "ones_sb")
            nc.sync.dma_start(out=sqd[:], in_=d_sqdeg[:])
            nc.sync.dma_start(out=bemb1[:], in_=d_bemb1[:])
            nc.sync.dma_start(out=bc[:], in_=d_bc[:])
            nc.sync.dma_start(out=bo[:], in_=d_bo[:])
            nc.gpsimd.memset(ones[:], 1.0)

        def piece_rows(b):
            # (piece, row0) of block b inside its agin piece
            if b < NB0:
                return 0, b * P
            return 1, (b - NB0) * P

        def launch_ag(i, p):
            nc.gpsimd.collective_compute(
                "AllGather",
                mybir.AluOpType.bypass,
                replica_groups=[list(range(NC))],
                ins=[agin[i][p][:]],
                outs=[agout[i][p][:]],
            )

        with tc.tile_pool(name="work", bufs=2) as wp, \
                tc.tile_pool(name="psum", bufs=2, space="PSUM") as pp:

            # ---- embedding transform (x @ W1x), feeds layer 0's AG
            xall = wp.tile([DIN, NPP], bf16, tag="xall", bufs=1)
            nc.sync.dma_start(out=xall[:], in_=d_xT[:])
            for b in range(NB):
                pt = pp.tile([P, DH], f32, tag="pt")
                bs = slice(b * P, (b + 1) * P)
                ds_ = slice(b * DH, (b + 1) * DH)
                nc.tensor.matmul(
                    out=pt[:], lhsT=xall[:, bs], rhs=w1x[:],
                    start=True, stop=not hb_emb,
                )
                if hb_emb:
                    nc.tensor.matmul(
                        out=pt[:], lhsT=ones[:, :], rhs=bemb1[:],
                        start=False, stop=True,
                    )
                nc.scalar.activation(
                    out=hwdall[:, ds_], in_=pt[:], func=COPY,
                    scale=dinv[:, b:b + 1],
                )
                p, r0 = piece_rows(b)
                nc.sync.dma_start(out=agin[0][p][r0:r0 + P, :],
                                  in_=hwdall[:, ds_])
                if b == NB0 - 1:
                    launch_ag(0, 0)
            launch_ag(0, 1)

            # ---- layers (aggregation fused with next-layer transform)
            qrr = [0]

            def gath(dst_tile, src_ap, cols, ntiles):
                for k0 in range(0, ntiles, STEP):
                    kt = min(STEP, ntiles - k0)
                    q = qrr[0] % 4
                    qrr[0] += 1
                    nc.gpsimd.dma_gather(
                        out_ap=dst_tile[:, k0 * DH:(k0 + kt) * DH]
                        .rearrange("p (t e) -> p t e", e=DH),
                        in_ap=src_ap,
                        idxs_ap=gidx[:, cols[0] + k0 * 8:cols[0] + (k0 + kt) * 8],
                        num_idxs=kt * P,
                        num_idxs_reg=kt * P,
                        elem_size=DH,
                        single_packet=False,
                        queue_num=q,
                    )

            for i in range(L):
                for ci, ch in enumerate(chunks):
                    tp0, tp1 = ch["tp0"], ch["tp1"]
                    nt = tp0 + tp1
                    if tp0 > 0:
                        msg0 = wp.tile([P, tp0 * DH], bf16, tag="msg0", bufs=2)
                        gath(msg0, agout[i][0][:], ch["cols0"], tp0)
                    if tp1 > 0:
                        msg1 = wp.tile([P, tp1 * DH], bf16, tag="msg1", bufs=2)
                        gath(msg1, agout[i][1][:], ch["cols1"], tp1)
                    # build one-hot selection tiles on-chip:
                    # smat[e, t*128+c] = (colcode[e, t] == c)
                    smt = wp.tile([P, nt * P], bf16, tag="smat", bufs=2)
                    t0 = ch["smat_tiles"][0]
                    nc.vector.tensor_tensor(
                        out=smt[:].rearrange("p (t c) -> p t c", c=P),
                        in0=colc[:, t0:t0 + nt].unsqueeze(2)
                        .broadcast_to((P, nt, P)),
                        in1=iota[:].unsqueeze(1).broadcast_to((P, nt, P)),
                        op=EQ,
                    )

                    for b in ch["blocks"]:
                        pa = pp.tile([P, DH], f32, tag="pa", bufs=4)
                        bs = slice(b * P, (b + 1) * P)
                        ds_ = slice(b * DH, (b + 1) * DH)
                        nmm = int(T[b, 0]) + int(T[b, 1]) + (1 if hb_conv else 0)
                        j = 0
                        for t in range(int(T[b, 0])):
                            s_pos = ch["base0"][b] + t
                            j += 1
                            nc.tensor.matmul(
                                out=pa[:],
                                lhsT=smt[:, s_pos * P:(s_pos + 1) * P],
                                rhs=msg0[:, s_pos * DH:(s_pos + 1) * DH],
                                start=(j == 1), stop=(j == nmm),
                            )
                        for t in range(int(T[b, 1])):
                            s_pos = ch["base1"][b] + t
                            j += 1
                            nc.tensor.matmul(
                                out=pa[:],
                                lhsT=smt[:, (tp0 + s_pos) * P:(tp0 + s_pos + 1) * P],
                                rhs=msg1[:, s_pos * DH:(s_pos + 1) * DH],
                                start=(j == 1), stop=(j == nmm),
                            )
                        if hb_conv:
                            j += 1
                            nc.tensor.matmul(
                                out=pa[:], lhsT=sqd[:, bs],
                                rhs=bc[:, i * DH:(i + 1) * DH],
                                start=(j == 1), stop=True,
                            )
                        assert j == nmm and nmm > 0
                        # self-loop term: selfk[d] * hwd_local[d, :]  (ACT)
                        sterm = wp.tile([P, DH], bf16, tag="sterm")
                        nc.scalar.activation(
                            out=sterm[:], in_=hwdall[:, ds_], func=COPY,
                            scale=selfk[:, b:b + 1],
                        )
                        psum = wp.tile([P, DH], f32, tag="psumadd")
                        nc.vector.tensor_tensor(
                            out=psum[:], in0=pa[:], in1=sterm[:], op=ADD,
                        )
                        hnew = wp.tile([P, DH], bf16, tag="hnew")
                        nc.scalar.activation(
                            out=hnew[:], in_=psum[:], func=TANH,
                            scale=dinv[:, b:b + 1],
                        )
                        for k, hT in enumerate((hT0, hT1)):
                            ptr = pp.tile([P, P], bf16, tag="ptr")
                            nc.tensor.transpose(
                                out=ptr[:], in_=hnew[:, k * P:(k + 1) * P],
                                identity=ident[:],
                            )
                            nc.vector.tensor_copy(out=hT[:, bs], in_=ptr[:])

                        if i < L - 1:
                            # transform for layer i+1, feed its AG pieces
                            pt = pp.tile([P, DH], f32, tag="pt")
                            nc.tensor.matmul(
                                out=pt[:], lhsT=hT0[:, bs],
                                rhs=wc[:, (2 * i) * DH:(2 * i + 1) * DH],
                                start=True, stop=False,
                            )
                            nc.tensor.matmul(
                                out=pt[:], lhsT=hT1[:, bs],
                                rhs=wc[:, (2 * i + 1) * DH:(2 * i + 2) * DH],
                                start=False, stop=True,
                            )
                            nc.scalar.activation(
                                out=hwdall[:, ds_], in_=pt[:], func=COPY,
                                scale=dinv[:, b:b + 1],
                            )
                            p, r0 = piece_rows(b)
                            nc.sync.dma_start(
                                out=agin[i + 1][p][r0:r0 + P, :],
                                in_=hwdall[:, ds_],
                            )
                            if b == NB0 - 1:
                                launch_ag(i + 1, 0)
                            elif b == NB - 1:
                                launch_ag(i + 1, 1)
                        else:
                            po = pp.tile([P, DOUT], f32, tag="pt")
                            nc.tensor.matmul(
                                out=po[:], lhsT=hT0[:, bs], rhs=wo[:, :DOUT],
                                start=True, stop=False,
                            )
                            nc.tensor.matmul(
                                out=po[:], lhsT=hT1[:, bs],
                                rhs=wo[:, DOUT:2 * DOUT],
                                start=False, stop=not hb_out,
                            )
                            if hb_out:
                                nc.tensor.matmul(
                                    out=po[:], lhsT=ones[:, :], rhs=bo[:],
                                    start=False, stop=True,
                                )
                            osb = wp.tile([P, DOUT], f32, tag="osb")
                            nc.scalar.activation(out=osb[:], in_=po[:], func=COPY)
                            rows = min(P, NPs - b * P)
                            nc.scalar.dma_start(
                                out=d_out[b * P:b * P + rows, :],
                                in_=osb[:rows, :],
                            )

        pers.release()
        dpool.release()

    nc.compile()
    return nc


# ----------------------------------------------------------------------------
# driver
# ----------------------------------------------------------------------------


def run(inputs, cfg, trace=False):
    from concourse import bass_utils

    NC, N = cfg["NC"], cfg["N"]
    in_maps, meta = preprocess(inputs, cfg)
    nc = build_program(meta, cfg)
    res = bass_utils.run_bass_kernel_spmd(
        nc, in_maps, core_ids=list(range(NC)), trace=trace,
    )
    out = np.concatenate([res.results[c]["out"] for c in range(NC)], axis=0)
    return np.ascontiguousarray(out[:N]).astype(np.float32), res


# revision 11
# speedup vs baseline: 1.0619x; 1.0619x over previous
"""GCN (4-layer message-passing) Trainium2 kernel, 8-core SPMD.

Math (matches PyG GCNConv with self-loops, per reference):
    deg[d]  = in-degree over (edges + self-loops)
    dinv    = deg^-1/2
    h0      = x @ W_emb + b_emb
    layer i: h <- tanh( dinv[d] * sum_{e: dst=d} dinv[src_e] * (h @ W_i)[src_e] + b_i )
    out     = h @ W_out + b_out

Distribution: nodes sharded across 8 cores (dst-sharded edges). Per layer:
  1. transform own shard:  hWd = dinv * (h @ W)   (PE matmul + ACT scale/cast bf16)
  2. AllGather hWd across cores in TWO row-pieces (piece 0 = blocks 0..24,
     piece 1 = blocks 25..48), so gathers of piece 0 overlap the piece-1
     collective, and the next layer's collectives launch mid-aggregation.
  3. dma_gather (SWDGE) each in-edge's source row, sorted by (dst blk, piece)
  4. segment-sum via PE matmuls against one-hot selection tiles built
     ON-CHIP (DVE is_equal against an iota ramp), accumulated in PSUM
  5. + self-loop term (DVE per-partition multiply of local hwd) [+ bias]
  6. tanh with per-partition dinv scale on ACT; PE-transpose back to h^T;
     immediately transform for the next layer and feed its AllGather.

The embedding layer is folded into layer 1's weights host-side.
Each AG piece's table has < 32768 rows so int16 gather indices address it
directly (no lo/hi split).
"""

import math

import ml_dtypes
import numpy as np

BF16 = ml_dtypes.bfloat16
P = 128

CFG_FULL = dict(N=50000, E=800000, DIN=128, DH=256, DOUT=64, L=4, NC=8)

CHUNK_BLOCKS = 2     # dst blocks per gather/aggregation chunk
STEP = 24            # max tiles (x128 idxs) per dma_gather call
PIECE0_NB = 25       # blocks 0..24 -> AG piece 0; rest -> piece 1
SCRATCH = 32768


def kernel(**inputs) -> np.ndarray:
    out, _ = run(inputs, CFG_FULL)
    return out


# ----------------------------------------------------------------------------
# host-side preprocessing
# ----------------------------------------------------------------------------


def _ceil_div(a, b):
    return (a + b - 1) // b


def preprocess(inputs, cfg):
    N, E, DIN, DH, DOUT, L, NC = (
        cfg["N"], cfg["E"], cfg["DIN"], cfg["DH"], cfg["DOUT"], cfg["L"], cfg["NC"],
    )
    x = np.asarray(inputs["x"], np.float32)
    ei = np.asarray(inputs["edge_index"]).astype(np.int64)
    W_emb = np.asarray(inputs["W_emb"], np.float32)
    b_emb = np.asarray(inputs["b_emb"], np.float32)
    W_conv = np.asarray(inputs["W_conv"], np.float32)
    b_conv = np.asarray(inputs["b_conv"], np.float32)
    W_out = np.asarray(inputs["W_out"], np.float32)
    b_out = np.asarray(inputs["b_out"], np.float32)

    deg = (np.bincount(ei[1], minlength=N) + 1).astype(np.float32)
    dinv = (1.0 / np.sqrt(np.maximum(deg, 1.0))).astype(np.float32)
    sqdeg = np.sqrt(np.maximum(deg, 1.0)).astype(np.float32)

    # self-edges (incl. the implicit self-loop): exact integer multiplicity
    # applied on-chip as a per-partition DVE multiply of the local hwd rows
    selfmask = ei[0] == ei[1]
    selfk = 1 + np.bincount(ei[1][selfmask], minlength=N)
    src = ei[0][~selfmask]
    dst = ei[1][~selfmask]

    NPs = _ceil_div(N, NC)          # real nodes per shard (6250)
    NB = _ceil_div(NPs, P)          # dst blocks per core (49)
    NPP = NB * P                    # padded nodes per shard (6272)
    NB0 = PIECE0_NB
    NB1 = NB - NB0
    PROWS = [NB0 * P, NB1 * P]      # piece rows per core
    AGR = [NC * PROWS[0], NC * PROWS[1]]
    assert max(AGR) < 32768

    # edge -> (piece, piece-local AG row)
    cs = src // NPs
    ls = src - cs * NPs
    piece = (ls >= PROWS[0]).astype(np.int64)
    prow = np.where(piece == 0, cs * PROWS[0] + ls,
                    cs * PROWS[1] + (ls - PROWS[0]))

    core_of = dst // NPs
    d_loc = dst - core_of * NPs
    blk = d_loc // P
    col = d_loc % P

    # per-core edge partitions, sorted by (block, piece, dstcol, srcrow)
    cores = []
    nseg = np.zeros((NC, NB, 2), np.int64)
    for c in range(NC):
        m = core_of == c
        a_blk, a_pc, a_col, a_row = blk[m], piece[m], col[m], prow[m]
        order = np.lexsort((a_row, a_col, a_pc, a_blk))
        a_blk, a_pc, a_col, a_row = (
            a_blk[order], a_pc[order], a_col[order], a_row[order],
        )
        cnt = np.bincount(a_blk * 2 + a_pc, minlength=NB * 2).reshape(NB, 2)
        nseg[c] = cnt
        cores.append((a_blk, a_pc, a_col, a_row))

    nmax = nseg.max(axis=0)                      # [NB, 2]
    T = _ceil_div(nmax, P)                       # tiles per (block, piece)

    # chunk layout (identical across cores)
    chunks = []
    gidx_col = 0
    tile_ctr = 0
    for g0 in range(0, NB, CHUNK_BLOCKS):
        blocks = list(range(g0, min(g0 + CHUNK_BLOCKS, NB)))
        tp0 = int(T[blocks, 0].sum())
        tp1 = int(T[blocks, 1].sum())
        cols0 = (gidx_col, gidx_col + tp0 * P // 16)
        gidx_col = cols0[1]
        cols1 = (gidx_col, gidx_col + tp1 * P // 16)
        gidx_col = cols1[1]
        base0, base1 = {}, {}
        t = 0
        for b in blocks:
            base0[b] = t
            t += int(T[b, 0])
        t = 0
        for b in blocks:
            base1[b] = t
            t += int(T[b, 1])
        smat_tiles = (tile_ctr, tile_ctr + tp0 + tp1)
        tile_ctr = smat_tiles[1]
        chunks.append(dict(
            blocks=blocks, tp0=tp0, tp1=tp1, cols0=cols0, cols1=cols1,
            base0=base0, base1=base1, smat_tiles=smat_tiles,
        ))
    GC = gidx_col
    TT = tile_ctr

    meta = dict(
        NPs=NPs, NB=NB, NPP=NPP, NB0=NB0, NB1=NB1, PROWS=PROWS, AGR=AGR,
        T=T, chunks=chunks, GC=GC, TT=TT,
        hb_emb=bool(np.any(b_emb @ W_conv[0])),
        hb_conv=bool(np.any(b_conv)),
        hb_out=bool(np.any(b_out)),
    )

    # shared weights
    W1x = (W_emb @ W_conv[0]).astype(BF16)                    # [DIN, DH]
    bemb1 = (b_emb @ W_conv[0]).reshape(1, DH).astype(BF16)
    Wc = W_conv[1:].reshape((L - 1) * DH, DH).astype(BF16) if L > 1 else \
        np.zeros((0, DH), BF16)
    bc = b_conv.reshape(1, L * DH).astype(BF16)
    Wo = W_out.astype(BF16)                                    # [DH, DOUT]
    bo = b_out.reshape(1, DOUT).astype(BF16)

    in_maps = []
    for c in range(NC):
        a_blk, a_pc, a_col, a_row = cores[c]
        n0 = c * NPs
        n1 = min(n0 + NPs, N)
        nreal = n1 - n0

        # per-edge slot in the chunk-ordered tile stream
        seg_id = a_blk * 2 + a_pc
        seg_start = np.zeros(NB * 2, np.int64)
        cnts = np.bincount(seg_id, minlength=NB * 2)
        seg_start[1:] = np.cumsum(cnts)[:-1]
        epos = np.arange(len(seg_id)) - seg_start[seg_id]

        tile_of_seg = np.zeros(NB * 2, np.int64)
        for ch in chunks:
            for b in ch["blocks"]:
                tile_of_seg[b * 2] = ch["smat_tiles"][0] + ch["base0"][b]
                tile_of_seg[b * 2 + 1] = (
                    ch["smat_tiles"][0] + ch["tp0"] + ch["base1"][b]
                )
        e_tile = tile_of_seg[seg_id] + epos // P
        e_row = epos % P

        # dst-column code per (tile, slot); -1 marks padding
        colcode = np.full((P, TT), -1, np.int16)
        colcode[e_row, e_tile] = a_col.astype(np.int16)

        # gather indices, wrapped layout [16->128, GC] int16
        gidx = np.zeros((16, GC), np.int16)
        for ch in chunks:
            for p, colrange, base_map, tcount in (
                (0, ch["cols0"], ch["base0"], ch["tp0"]),
                (1, ch["cols1"], ch["base1"], ch["tp1"]),
            ):
                if tcount == 0:
                    continue
                vals = np.zeros(tcount * P, np.int64)
                for b in ch["blocks"]:
                    m = (a_blk == b) & (a_pc == p)
                    v = a_row[m]
                    off = base_map[b] * P
                    vals[off:off + len(v)] = v
                c0, c1 = colrange
                gidx[:, c0:c1] = vals.reshape(c1 - c0, 16).T
        gidx = np.tile(gidx, (8, 1)).astype(np.int16)

        # dinv [128, NB] fp32 ; selfk [128, NB] fp32 ; sqdeg [1, NPP] bf16
        dl = np.ones(NPP, np.float32)
        dl[:nreal] = dinv[n0:n1]
        dinvp = dl.reshape(NB, P).T.copy()
        kk = np.zeros(NPP, np.float32)
        kk[:nreal] = selfk[n0:n1]
        selfkp = kk.reshape(NB, P).T.copy()
        sq = np.ones(NPP, np.float32)
        sq[:nreal] = sqdeg[n0:n1]
        sqdegp = sq.reshape(1, NPP).astype(BF16)

        xT = np.zeros((DIN, NPP), BF16)
        xT[:, :nreal] = x[n0:n1].T

        in_maps.append(dict(
            xT=np.ascontiguousarray(xT),
            gidx=np.ascontiguousarray(gidx),
            colcode=np.ascontiguousarray(colcode),
            dinvp=np.ascontiguousarray(dinvp),
            selfkp=np.ascontiguousarray(selfkp),
            sqdegp=np.ascontiguousarray(sqdegp),
            w1x=W1x, bemb1=bemb1, wc=Wc, bc=bc, wo=Wo, bo=bo,
        ))

    return in_maps, meta


# ----------------------------------------------------------------------------
# device program
# ----------------------------------------------------------------------------


def build_program(meta, cfg):
    import concourse.bacc as bacc
    import concourse.mybir as mybir
    import concourse.tile as tile
    from concourse.masks import make_identity

    N, DIN, DH, DOUT, L, NC = (
        cfg["N"], cfg["DIN"], cfg["DH"], cfg["DOUT"], cfg["L"], cfg["NC"],
    )
    NPs, NB, NPP, NB0, PROWS, AGR = (
        meta["NPs"], meta["NB"], meta["NPP"], meta["NB0"], meta["PROWS"],
        meta["AGR"],
    )
    T, chunks, GC, TT = meta["T"], meta["chunks"], meta["GC"], meta["TT"]
    hb_emb, hb_conv, hb_out = meta["hb_emb"], meta["hb_conv"], meta["hb_out"]
    assert DIN == P

    f32 = mybir.dt.float32
    bf16 = mybir.dt.bfloat16
    i16 = mybir.dt.int16
    TANH = mybir.ActivationFunctionType.Tanh
    COPY = mybir.ActivationFunctionType.Copy
    EQ = mybir.AluOpType.is_equal
    MUL = mybir.AluOpType.mult
    ADD = mybir.AluOpType.add

    nc = bacc.Bacc("TRN2", target_bir_lowering=False, debug=False, num_devices=NC,
                   num_swdge_queues=4, dynamic_dma_scratch_size=SCRATCH)

    # I/O
    d_xT = nc.dram_tensor("xT", [DIN, NPP], bf16, kind="ExternalInput")
    d_gidx = nc.dram_tensor("gidx", [P, GC], i16, kind="ExternalInput")
    d_colc = nc.dram_tensor("colcode", [P, TT], i16, kind="ExternalInput")
    d_dinv = nc.dram_tensor("dinvp", [P, NB], f32, kind="ExternalInput")
    d_selfk = nc.dram_tensor("selfkp", [P, NB], f32, kind="ExternalInput")
    d_sqdeg = nc.dram_tensor("sqdegp", [1, NPP], bf16, kind="ExternalInput")
    d_w1x = nc.dram_tensor("w1x", [DIN, DH], bf16, kind="ExternalInput")
    d_bemb1 = nc.dram_tensor("bemb1", [1, DH], bf16, kind="ExternalInput")
    d_wc = nc.dram_tensor("wc", [(L - 1) * DH, DH], bf16, kind="ExternalInput")
    d_bc = nc.dram_tensor("bc", [1, L * DH], bf16, kind="ExternalInput")
    d_wo = nc.dram_tensor("wo", [DH, DOUT], bf16, kind="ExternalInput")
    d_bo = nc.dram_tensor("bo", [1, DOUT], bf16, kind="ExternalInput")
    d_out = nc.dram_tensor("out", [NPs, DOUT], f32, kind="ExternalOutput")

    with tile.TileContext(nc) as tc:
        pers = tc.alloc_tile_pool(name="pers", bufs=1)
        dpool = tc.alloc_tile_pool(name="dpers", bufs=1, space="DRAM")

        # internal DRAM for the collectives: [layer][piece]
        agin = [
            [dpool.tile([PROWS[p], DH], bf16, name=f"agin{i}_{p}",
                        tag=f"agin{i}_{p}") for p in range(2)]
            for i in range(L)
        ]
        agout = [
            [dpool.tile([AGR[p], DH], bf16, name=f"agout{i}_{p}",
                        tag=f"agout{i}_{p}", addr_space="Shared")
             for p in range(2)]
            for i in range(L)
        ]

        def stile(shape, dtype, name):
            return pers.tile(shape, dtype, name=name, tag=name)

        # persistent SBUF state
        hT0 = stile([P, NPP], bf16, "hT0_sb")
        hT1 = stile([P, NPP], bf16, "hT1_sb")
        hwdall = stile([P, NB * DH], bf16, "hwdall_sb")
        gidx = stile([P, GC], i16, "gidx_sb")
        colc = stile([P, TT], i16, "colc_sb")
        dinv = stile([P, NB], f32, "dinv_sb")
        selfk = stile([P, NB], f32, "selfk_sb")
        w1x = stile([DIN, DH], bf16, "w1x_sb")
        wc = stile([P, 2 * (L - 1) * DH], bf16, "wc_sb")
        wo = stile([P, 2 * DOUT], bf16, "wo_sb")
        ident = stile([P, P], bf16, "ident_sb")
        iota = stile([P, P], i16, "iota_sb")

        nc.sync.dma_start(out=gidx[:], in_=d_gidx[:])
        nc.sync.dma_start(out=colc[:], in_=d_colc[:])
        nc.sync.dma_start(out=dinv[:], in_=d_dinv[:])
        nc.sync.dma_start(out=selfk[:], in_=d_selfk[:])
        nc.sync.dma_start(out=w1x[:], in_=d_w1x[:])
        for i in range(L - 1):
            for k in range(2):
                nc.sync.dma_start(
                    out=wc[:, (2 * i + k) * DH:(2 * i + k + 1) * DH],
                    in_=d_wc[i * DH + k * P:i * DH + (k + 1) * P, :],
                )
        for k in range(2):
            nc.sync.dma_start(
                out=wo[:, k * DOUT:(k + 1) * DOUT],
                in_=d_wo[k * P:(k + 1) * P, :],
            )
        make_identity(nc, ident[:])
        # iota row 0..127 along the free dim, same on every partition
        nc.gpsimd.iota(iota[:], pattern=[[1, P]], base=0, channel_multiplier=0)
        any_bias = hb_emb or hb_conv or hb_out
        if any_bias:
            sqd = stile([1, NPP], bf16, "sqdeg_sb")
            bemb1 = stile([1, DH], bf16, "bemb1_sb")
            bc = stile([1, L * DH], bf16, "bc_sb")
            bo = stile([1, DOUT], bf16, "bo_sb")
            ones = stile([1, P], bf16, "ones_sb")
            nc.sync.dma_start(out=sqd[:], in_=d_sqdeg[:])
            nc.sync.dma_start(out=bemb1[:], in_=d_bemb1[:])
            nc.sync.dma_start(out=bc[:], in_=d_bc[:])
            nc.sync.dma_start(out=bo[:], in_=d_bo[:])
            nc.gpsimd.memset(ones[:], 1.0)

        def piece_rows(b):
            # (piece, row0) of block b inside its agin piece
            if b < NB0:
                return 0, b * P
            return 1, (b - NB0) * P

        def launch_ag(i, p):
            nc.gpsimd.collective_compute(
                "AllGather",
                mybir.AluOpType.bypass,
                replica_groups=[list(range(NC))],
                ins=[agin[i][p][:]],
                outs=[agout[i][p][:]],
            )

        with tc.tile_pool(name="work", bufs=2) as wp, \
                tc.tile_pool(name="psum", bufs=2, space="PSUM") as pp:

            # ---- embedding transform (x @ W1x), feeds layer 0's AG
            xall = wp.tile([DIN, NPP], bf16, tag="xall", bufs=1)
            nc.sync.dma_start(out=xall[:], in_=d_xT[:])
            for b in range(NB):
                pt = pp.tile([P, DH], f32, tag="pt")
                bs = slice(b * P, (b + 1) * P)
                ds_ = slice(b * DH, (b + 1) * DH)
                nc.tensor.matmul(
                    out=pt[:], lhsT=xall[:, bs], rhs=w1x[:],
                    start=True, stop=not hb_emb,
                )
                if hb_emb:
                    nc.tensor.matmul(
                        out=pt[:], lhsT=ones[:, :], rhs=bemb1[:],
                        start=False, stop=True,
                    )
                nc.scalar.activation(
                    out=hwdall[:, ds_], in_=pt[:], func=COPY,
                    scale=dinv[:, b:b + 1],
                )
                p, r0 = piece_rows(b)
                nc.sync.dma_start(out=agin[0][p][r0:r0 + P, :],
                                  in_=hwdall[:, ds_])
                if b == NB0 - 1:
                    launch_ag(0, 0)
            launch_ag(0, 1)

            # ---- layers (aggregation fused with next-layer transform)
            qrr = [0]

            def gath(dst_tile, src_ap, cols, ntiles):
                for k0 in range(0, ntiles, STEP):
                    kt = min(STEP, ntiles - k0)
                    q = qrr[0] % 4
                    qrr[0] += 1
                    nc.gpsimd.dma_gather(
                        out_ap=dst_tile[:, k0 * DH:(k0 + kt) * DH]
                        .rearrange("p (t e) -> p t e", e=DH),
                        in_ap=src_ap,
                        idxs_ap=gidx[:, cols[0] + k0 * 8:cols[0] + (k0 + kt) * 8],
                        num_idxs=kt * P,
                        num_idxs_reg=kt * P,
                        elem_size=DH,
                        single_packet=False,
                        queue_num=q,
                    )

            for i in range(L):
                for ci, ch in enumerate(chunks):
                    tp0, tp1 = ch["tp0"], ch["tp1"]
                    nt = tp0 + tp1
                    if tp0 > 0:
                        msg0 = wp.tile([P, tp0 * DH], bf16, tag="msg0", bufs=3)
                        gath(msg0, agout[i][0][:], ch["cols0"], tp0)
                    if tp1 > 0:
                        msg1 = wp.tile([P, tp1 * DH], bf16, tag="msg1", bufs=3)
                        gath(msg1, agout[i][1][:], ch["cols1"], tp1)
                    # build one-hot selection tiles on-chip:
                    # smat[e, t*128+c] = (colcode[e, t] == c)
                    smt = wp.tile([P, nt * P], bf16, tag="smat", bufs=3)
                    t0 = ch["smat_tiles"][0]
                    nc.vector.tensor_tensor(
                        out=smt[:].rearrange("p (t c) -> p t c", c=P),
                        in0=colc[:, t0:t0 + nt].unsqueeze(2)
                        .broadcast_to((P, nt, P)),
                        in1=iota[:].unsqueeze(1).broadcast_to((P, nt, P)),
                        op=EQ,
                    )

                    for b in ch["blocks"]:
                        pa = pp.tile([P, DH], f32, tag="pa", bufs=4)
                        bs = slice(b * P, (b + 1) * P)
                        ds_ = slice(b * DH, (b + 1) * DH)
                        nmm = int(T[b, 0]) + int(T[b, 1]) + (1 if hb_conv else 0)
                        j = 0
                        for t in range(int(T[b, 0])):
                            s_pos = ch["base0"][b] + t
                            j += 1
                            nc.tensor.matmul(
                                out=pa[:],
                                lhsT=smt[:, s_pos * P:(s_pos + 1) * P],
                                rhs=msg0[:, s_pos * DH:(s_pos + 1) * DH],
                                start=(j == 1), stop=(j == nmm),
                            )
                        for t in range(int(T[b, 1])):
                            s_pos = ch["base1"][b] + t
                            j += 1
                            nc.tensor.matmul(
                                out=pa[:],
                                lhsT=smt[:, (tp0 + s_pos) * P:(tp0 + s_pos + 1) * P],
                                rhs=msg1[:, s_pos * DH:(s_pos + 1) * DH],
                                start=(j == 1), stop=(j == nmm),
                            )
                        if hb_conv:
                            j += 1
                            nc.tensor.matmul(
                                out=pa[:], lhsT=sqd[:, bs],
                                rhs=bc[:, i * DH:(i + 1) * DH],
                                start=(j == 1), stop=True,
                            )
                        assert j == nmm and nmm > 0
                        # self-loop term: selfk[d] * hwd_local[d, :]  (ACT)
                        sterm = wp.tile([P, DH], bf16, tag="sterm")
                        nc.scalar.activation(
                            out=sterm[:], in_=hwdall[:, ds_], func=COPY,
                            scale=selfk[:, b:b + 1],
                        )
                        psum = wp.tile([P, DH], f32, tag="psumadd")
                        nc.vector.tensor_tensor(
                            out=psum[:], in0=pa[:], in1=sterm[:], op=ADD,
                        )
                        hnew = wp.tile([P, DH], bf16, tag="hnew")
                        nc.scalar.activation(
                            out=hnew[:], in_=psum[:], func=TANH,
                            scale=dinv[:, b:b + 1],
                        )
                        for k, hT in enumerate((hT0, hT1)):
                            ptr = pp.tile([P, P], bf16, tag="ptr")
                            nc.tensor.transpose(
                                out=ptr[:], in_=hnew[:, k * P:(k + 1) * P],
                                identity=ident[:],
                            )
                            nc.vector.tensor_copy(out=hT[:, bs], in_=ptr[:])

                        if i < L - 1:
                            # transform for layer i+1, feed its AG pieces
                            pt = pp.tile([P, DH], f32, tag="pt")
                            nc.tensor.matmul(
                                out=pt[:], lhsT=hT0[:, bs],
                                rhs=wc[:, (2 * i) * DH:(2 * i + 1) * DH],
                                start=True, stop=False,
                            )
                            nc.tensor.matmul(
                                out=pt[:], lhsT=hT1[:, bs],
                                rhs=wc[:, (2 * i + 1) * DH:(2 * i + 2) * DH],
                                start=False, stop=True,
                            )
                            nc.scalar.activation(
                                out=hwdall[:, ds_], in_=pt[:], func=COPY,
                                scale=dinv[:, b:b + 1],
                            )
                            p, r0 = piece_rows(b)
                            nc.sync.dma_start(
                                out=agin[i + 1][p][r0:r0 + P, :],
                                in_=hwdall[:, ds_],
                            )
                            if b == NB0 - 1:
                                launch_ag(i + 1, 0)
                            elif b == NB - 1:
                                launch_ag(i + 1, 1)
                        else:
                            po = pp.tile([P, DOUT], f32, tag="pt")
                            nc.tensor.matmul(
                                out=po[:], lhsT=hT0[:, bs], rhs=wo[:, :DOUT],
                                start=True, stop=False,
                            )
                            nc.tensor.matmul(
                                out=po[:], lhsT=hT1[:, bs],
                                rhs=wo[:, DOUT:2 * DOUT],
                                start=False, stop=not hb_out,
                            )
                            if hb_out:
                                nc.tensor.matmul(
                                    out=po[:], lhsT=ones[:, :], rhs=bo[:],
                                    start=False, stop=True,
                                )
                            osb = wp.tile([P, DOUT], f32, tag="osb")
                            nc.scalar.activation(out=osb[:], in_=po[:], func=COPY)
                            rows = min(P, NPs - b * P)
                            nc.scalar.dma_start(
                                out=d_out[b * P:b * P + rows, :],
                                in_=osb[:rows, :],
                            )

        pers.release()
        dpool.release()

    nc.compile()
    return nc


# ----------------------------------------------------------------------------
# driver
# ----------------------------------------------------------------------------


def run(inputs, cfg, trace=False):
    from concourse import bass_utils

    NC, N = cfg["NC"], cfg["N"]
    in_maps, meta = preprocess(inputs, cfg)
    nc = build_program(meta, cfg)
    res = bass_utils.run_bass_kernel_spmd(
        nc, in_maps, core_ids=list(range(NC)), trace=trace,
    )
    out = np.concatenate([res.results[c]["out"] for c in range(NC)], axis=0)
    return np.ascontiguousarray(out[:N]).astype(np.float32), res
